# revision 5
# baseline (speedup 1.0000x reference)
"""Trainium2 Bass kernel for nn_Conv1dFFTInt8.

The reference computes, per (b, o):
    out[b,o,0] = ifft(fft(x) . fft(w) summed over cin)[0] + bias[o]
By the circular correlation theorem this collapses to a plain dot product:
    out[b,o] = sum_{i,n} x[b,i,n] * w[o,i,(L-n) % L] + bias[o]

So the whole problem is a GEMM: [B, CIN*L] @ [CIN*L, COUT] with a 524288-deep
contraction. We shard the contraction (CIN) across 8 cores (16 channels each).

v2 (DoubleRow): each core runs 256 fp8e4 DoubleRow matmuls. DoubleRow
processes TWO 128-deep k-tiles per instruction (the PE holds two weight rows
per cell and double-pumps the moving stream), so the PE consumes a pair of
k-tiles in ~53ns instead of ~107ns - the PE drops off the critical path and
the kernel is purely DMA-stream-bound.

DoubleRow requires BOTH operands in fp8e4/e5. x in e4m3 alone costs rel err
2.6e-2 (> the 2e-2 gate), so the stationary operand packs an e4m3 RESIDUAL
correction into the idle M columns: lhsT = [K=128, 2, 32] where cols 0-15
are x_hi = e4m3(x) and cols 16-31 are x_r = e4m3(x - x_hi). One w stream
then computes main + correction simultaneously (PSUM rows 16-31 hold the
correction); host sums the halves. Measured rel err: 7.4e-4.

Streams per core: w 8.39 MB + x 2.10 MB (hi+residual) = 10.5 MB against a
~360-420 GB/s per-core HBM stream, chunked at the 4KB-per-partition
descriptor knee and byte-balanced across the two HWDGE rings (sync/scalar).
Four PE column strips (tile_position cols 0/32/64/96) accumulate in separate
PSUM banks so LDWEIGHTS hides under other strips' matmuls. Warmup dummy MMs
pre-trip the HAM clock up during the (fixed) NEFF preamble; light keepalives
hold it there across chunk waits. Tail: DVE+ACT evacuate strips in parallel,
one 64 KB out DMA whose receipt is not waited (NEFF-end drains cover it).
"""

import numpy as np
import ml_dtypes

import concourse.bass as bass
from concourse import bacc
import concourse.mybir as mybir
from concourse.bass_utils import run_bass_kernel_spmd

B, CIN, COUT, L = 16, 128, 128, 4096
NCORES = 8
CIN_SH = CIN // NCORES          # 16 channels per core
KT = 128                        # contraction depth per k-tile
NKT = CIN_SH * L // KT          # 512 k-tiles per core
NPAIR = NKT // 2                # 256 DoubleRow pairs per core
MST = 2 * B                     # stationary M columns: 16 hi + 16 residual

# --- tunables (A/B config) ---
CFG = dict(
    # DoubleRow ISA restriction (NeuronVerifier check_dual_fp8_restriction):
    # dst.start_partition must be 0, which in bass ties the stationary tile
    # to PE columns 0-31 - so no multi-strip tile_position in DR mode.
    # Multiple PSUM banks at the same partitions do not add LDW overlap, so
    # nstrip stays 1 unless measurement says LDW stalls the pipe.
    nstrip=1,                   # PE column strips (tile_position cols 32*s)
    # (pairs, ring) per w DMA chunk; ring 0=sync, 1=scalar. 16 pairs =
    # 4KB/partition (the descriptor-efficiency knee). Tail chunks ride a
    # single ring so the other queue finishes early and the loner drains at
    # the full aggregate SDMA rate.
    w_sched=((16, 1), (16, 0), (16, 1), (16, 0), (16, 1), (16, 0), (16, 1),
             (16, 0), (16, 1), (16, 0), (16, 1), (16, 0), (16, 1), (16, 0),
             (16, 0), (16, 0)),
    # (pairs, ring) per x DMA chunk; 64 pairs = 4KB/partition.
    x_sched=((64, 0), (64, 1), (64, 1), (64, 1)),
    warmup=60,                  # dummy MMs at PE start to pre-trip HAM
    # dummy MMs before each w chunk wait (index = chunk): fill PE idle while
    # waiting on DMA so HAM holds the clock up; none on the tail where the
    # PE may be behind.
    keepalive=(0, 0, 0, 0, 0, 0, 0, 0, 0, 0, 0, 0, 0, 0, 0, 0),
    wait_out=False,             # skip waiting for the out DMA receipt
)

TRACE = False                   # set by test.py to profile
LAST_RESULTS = None             # BassKernelResults of the last run

_PROG_CACHE = {}


def _build_program_raw(cfg):
    """Raw bacc implementation: manual semaphores, no TileContext."""
    nstrip = cfg["nstrip"]
    w_sched = cfg["w_sched"]
    x_sched = cfg["x_sched"]
    assert sum(c for c, _ in w_sched) == NPAIR
    assert sum(c for c, _ in x_sched) == NPAIR
    n_wc = len(w_sched)
    n_xc = len(x_sched)
    w_start = np.cumsum([0] + [c for c, _ in w_sched])  # pair offsets
    x_start = np.cumsum([0] + [c for c, _ in x_sched])
    # x chunk index needed before starting w chunk c
    x_need = [int(np.searchsorted(x_start, w_start[c + 1], side="left")) - 1
              for c in range(n_wc)]

    first_p = {j: j for j in range(nstrip)}
    last_p = {j: NPAIR - nstrip + j for j in range(nstrip)}

    nc = bacc.Bacc("TRN2", target_bir_lowering=False, debug=False,
                   num_devices=NCORES)
    xt_d = nc.dram_tensor("xt", [KT, NPAIR, 2, MST], mybir.dt.float8e4,
                          kind="ExternalInput")
    wt_d = nc.dram_tensor("wt", [KT, NPAIR, 2, COUT], mybir.dt.float8e4,
                          kind="ExternalInput")
    out_d = nc.dram_tensor("out", [KT, COUT], mybir.dt.float32,
                           kind="ExternalOutput")

    import contextlib
    with contextlib.ExitStack() as stack:
        ec = stack.enter_context
        # one sem per DMA transfer: with several transfers in flight on the
        # 16 SDMA engines, a single cumulative sem is unsound (fast engines
        # can reach 16*(c+1) before a slow engine lands transfer c).
        s_wc = [ec(nc.semaphore(f"s_w{c}")) for c in range(n_wc)]
        s_xc = [ec(nc.semaphore(f"s_x{c}")) for c in range(n_xc)]
        s_mm = ec(nc.semaphore("s_mm"))
        s_cp = ec(nc.semaphore("s_cp"))
        s_out = ec(nc.semaphore("s_out"))
        xs = ec(nc.sbuf_tensor("xs", [KT, NPAIR, 2, MST], mybir.dt.float8e4))
        ws = ec(nc.sbuf_tensor("ws", [KT, NPAIR, 2, COUT], mybir.dt.float8e4))
        osb = ec(nc.sbuf_tensor("osb", [KT, COUT], mybir.dt.float32))
        accs = [ec(nc.psum_tensor(f"acc{s}", [KT, COUT], mybir.dt.float32))
                for s in range(nstrip)]
        if cfg["warmup"] or any(cfg["keepalive"]):
            junk = ec(nc.sbuf_tensor("junk", [KT, COUT], mybir.dt.float8e4))
            scr = ec(nc.psum_tensor("scr", [KT, COUT], mybir.dt.float32))

        # per-ring issue list: (pair_start, kind, chunk_idx); x sorts ahead
        # of w at equal k so the stationary operand is always resident first.
        issues = {0: [], 1: []}
        for c, (chunk, r) in enumerate(x_sched):
            issues[r].append((int(x_start[c]), 0, c))
        for c, (chunk, r) in enumerate(w_sched):
            issues[r].append((int(w_start[c]), 1, c))
        for r in issues:
            issues[r].sort()

        def emit_ring(eng, ring):
            for _p0, kind, c in issues[ring]:
                if kind == 0:
                    a, b = int(x_start[c]), int(x_start[c + 1])
                    eng.dma_start(xs[:, a:b, :, :],
                                  xt_d[:, a:b, :, :]).then_inc(s_xc[c], 16)
                else:
                    a, b = int(w_start[c]), int(w_start[c + 1])
                    eng.dma_start(ws[:, a:b, :, :],
                                  wt_d[:, a:b, :, :]).then_inc(s_wc[c], 16)

        with nc.Block() as block:

            @block.sync
            def _(sync):
                emit_ring(sync, 0)

            @block.scalar
            def _(scalar):
                emit_ring(scalar, 1)
                # tail: evacuate the upper strips in parallel with DVE's,
                # then issue the out DMA right here (scalar is HWDGE too) -
                # saves the cross-engine wake of handing it back to sync.
                scalar.wait_ge(s_mm, 1)
                for s in range((nstrip + 1) // 2, nstrip):
                    scalar.copy(osb[32 * s:32 * s + 32, :],
                                accs[s][32 * s:32 * s + 32, :])
                scalar.wait_ge(s_cp, 1)
                nrow = 32 * nstrip
                scalar.dma_start(out_d[0:nrow, :],
                                 osb[0:nrow, :]).then_inc(s_out, 16)
                if cfg["wait_out"]:
                    scalar.wait_ge(s_out, 16)

            @block.tensor
            def _(tensor):
                def dummy_mms(n):
                    # scratch-bank matmuls: keep the PE busy across DMA waits
                    # so HAM holds the clock up; results are never read
                    for _ in range(n):
                        tensor.matmul(scr[0:B, :], junk[:, 0:B],
                                      junk[:, 0:COUT], start=True, stop=True)

                dummy_mms(cfg["warmup"])
                x_waited = -1
                ka = cfg["keepalive"]
                assert len(ka) == n_wc
                for c, (chunk, _r) in enumerate(w_sched):
                    dummy_mms(ka[c])
                    tensor.wait_ge(s_wc[c], 16)
                    if x_need[c] > x_waited:
                        x_waited = x_need[c]
                        tensor.wait_ge(s_xc[x_waited], 16)
                    for j in range(chunk):
                        p = int(w_start[c]) + j
                        s = p % nstrip
                        mm = tensor.matmul(
                            accs[s][32 * s:32 * s + 32, :],
                            xs[:, p, :, :],
                            ws[:, p, :, :],
                            start=(p == first_p[s]),
                            stop=(p == last_p[s]),
                            perf_mode=mybir.MatmulPerfMode.DoubleRow,
                            tile_position=(0, 32 * s),
                        )
                        if p == NPAIR - 1:
                            mm.then_inc(s_mm, 2)

            @block.vector
            def _(vector):
                vector.wait_ge(s_mm, 1)
                nlow = (nstrip + 1) // 2
                for s in range(nlow):
                    cp = vector.tensor_copy(
                        osb[32 * s:32 * s + 32, :],
                        accs[s][32 * s:32 * s + 32, :],
                    )
                    if s == nlow - 1:
                        cp.then_inc(s_cp, 1)

    nc.compile()
    return nc


def _get_program(cfg):
    key = repr(sorted(cfg.items()))
    if key not in _PROG_CACHE:
        _PROG_CACHE[key] = _build_program_raw(cfg)
    return _PROG_CACHE[key]


def _pack_operand(arr_k_major, ncols):
    """[K_total, ncols] contraction-major -> [KT, NPAIR, 2, ncols] where
    sb[r, p, i, c] = arr[(2p + i)*KT + r, c]."""
    a = arr_k_major.reshape(NKT, KT, ncols).transpose(1, 0, 2)
    return np.ascontiguousarray(a).reshape(KT, NPAIR, 2, ncols)


def kernel(x, weight, bias):
    import os
    if not TRACE:
        # profiling needs an NTFF hook this image lacks; never trace here
        os.environ["BASS_NEVER_TRACE"] = "1"
    else:
        os.environ.pop("BASS_NEVER_TRACE", None)
    x = np.asarray(x, dtype=np.float32)
    weight = np.asarray(weight, dtype=np.float32)
    bias = np.asarray(bias, dtype=np.float32)

    cfg = dict(CFG)
    nc = _get_program(cfg)
    nstrip = cfg["nstrip"]

    # w_rev[o,i,n] = weight[o,i,(L-n) % L]
    idx = (L - np.arange(L)) % L
    wrev = weight[:, :, idx]

    # split x into e4m3 hi + e4m3 residual (DoubleRow needs fp8e4 operands;
    # hi alone would cost 2.6e-2 rel err, hi+residual costs 7.4e-4)
    x_hi8 = x.astype(ml_dtypes.float8_e4m3)
    x_r8 = (x - x_hi8.astype(np.float32)).astype(ml_dtypes.float8_e4m3)

    in_maps = []
    for c in range(NCORES):
        i0 = c * CIN_SH
        wsl = wrev[:, i0:i0 + CIN_SH, :].reshape(COUT, CIN_SH * L)
        wt = _pack_operand(wsl.T.astype(ml_dtypes.float8_e4m3), COUT)
        # [K_total, 32] = hi cols 0-15, residual cols 16-31
        xh = x_hi8[:, i0:i0 + CIN_SH, :].reshape(B, CIN_SH * L).T
        xr = x_r8[:, i0:i0 + CIN_SH, :].reshape(B, CIN_SH * L).T
        xsl = np.concatenate([xh, xr], axis=1)
        xt = _pack_operand(xsl, MST)
        in_maps.append({"xt": xt, "wt": wt})

    global LAST_RESULTS
    res = run_bass_kernel_spmd(nc, in_maps, core_ids=list(range(NCORES)),
                               trace=TRACE)
    LAST_RESULTS = res

    acc = np.zeros((B, COUT), np.float32)
    for c in range(NCORES):
        o = res.results[c]["out"]
        for s in range(nstrip):
            acc += o[32 * s:32 * s + B, :]
            acc += o[32 * s + B:32 * s + 2 * B, :]
    out = acc + bias[None, :]
    return out[:, :, None].astype(np.float32)


# revision 6
# speedup vs baseline: 1.0324x; 1.0324x over previous
"""Trainium2 Bass kernel for nn_Conv1dFFTInt8.

The reference computes, per (b, o):
    out[b,o,0] = ifft(fft(x) . fft(w) summed over cin)[0] + bias[o]
By the circular correlation theorem this collapses to a plain dot product:
    out[b,o] = sum_{i,n} x[b,i,n] * w[o,i,(L-n) % L] + bias[o]

So the whole problem is a GEMM: [B, CIN*L] @ [CIN*L, COUT] with a 524288-deep
contraction. We shard the contraction (CIN) across 8 cores (16 channels each).

v2 (DoubleRow): each core runs 256 fp8e4 DoubleRow matmuls. DoubleRow
processes TWO 128-deep k-tiles per instruction (the PE holds two weight rows
per cell and double-pumps the moving stream), so the PE consumes a pair of
k-tiles in ~53ns instead of ~107ns - the PE drops off the critical path and
the kernel is purely DMA-stream-bound.

DoubleRow requires BOTH operands in fp8e4/e5. x in e4m3 alone costs rel err
2.6e-2 (> the 2e-2 gate), so the stationary operand packs an e4m3 RESIDUAL
correction into the idle M columns: lhsT = [K=128, 2, 32] where cols 0-15
are x_hi = e4m3(x) and cols 16-31 are x_r = e4m3(x - x_hi). One w stream
then computes main + correction simultaneously (PSUM rows 16-31 hold the
correction); host sums the halves. Measured rel err: 7.4e-4.

Streams per core: w 8.39 MB + x 2.10 MB (hi+residual) = 10.5 MB against a
~360-420 GB/s per-core HBM stream, chunked at the 4KB-per-partition
descriptor knee and byte-balanced across the two HWDGE rings (sync/scalar).
Four PE column strips (tile_position cols 0/32/64/96) accumulate in separate
PSUM banks so LDWEIGHTS hides under other strips' matmuls. Warmup dummy MMs
pre-trip the HAM clock up during the (fixed) NEFF preamble; light keepalives
hold it there across chunk waits. Tail: DVE+ACT evacuate strips in parallel,
one 64 KB out DMA whose receipt is not waited (NEFF-end drains cover it).
"""

import numpy as np
import ml_dtypes

import concourse.bass as bass
from concourse import bacc
import concourse.mybir as mybir
from concourse.bass_utils import run_bass_kernel_spmd

B, CIN, COUT, L = 16, 128, 128, 4096
NCORES = 8
CIN_SH = CIN // NCORES          # 16 channels per core
KT = 128                        # contraction depth per k-tile
NKT = CIN_SH * L // KT          # 512 k-tiles per core
NPAIR = NKT // 2                # 256 DoubleRow pairs per core
MST = 2 * B                     # stationary M columns: 16 hi + 16 residual

# --- tunables (A/B config) ---
CFG = dict(
    # DoubleRow ISA restriction (NeuronVerifier check_dual_fp8_restriction):
    # dst.start_partition must be 0, which in bass ties the stationary tile
    # to PE columns 0-31 - so no multi-strip tile_position in DR mode.
    # Multiple PSUM banks at the same partitions do not add LDW overlap, so
    # nstrip stays 1 unless measurement says LDW stalls the pipe.
    nstrip=1,                   # PE column strips (tile_position cols 32*s)
    # (pairs, ring) per w DMA chunk; ring 0=sync, 1=scalar. 16 pairs =
    # 4KB/partition (the descriptor-efficiency knee). Tail chunks ride a
    # single ring so the other queue finishes early and the loner drains at
    # the full aggregate SDMA rate.
    w_sched=((16, 1), (16, 0), (16, 1), (16, 0), (16, 1), (16, 0), (16, 1),
             (16, 0), (16, 1), (16, 0), (16, 1), (16, 0), (16, 1), (16, 0),
             (16, 0), (16, 0)),
    # (pairs, ring) per x DMA chunk; 64 pairs = 4KB/partition.
    x_sched=((64, 0), (64, 1), (64, 1), (64, 1)),
    warmup=60,                  # dummy MMs at PE start to pre-trip HAM
    # dummy MMs before each w chunk wait (index = chunk): fill PE idle while
    # waiting on DMA so HAM holds the clock up; none on the tail where the
    # PE may be behind.
    # HAM evaluates PE utilization in ~3.4us windows and halves the clock
    # (k=4) when it sees idle; at k=4 the PE cadence (107ns/pair) falls
    # behind the DMA stream (82ns/pair). Dummies fill the per-chunk waits to
    # hold k=8; none on the tail where the PE may be genuinely behind.
    keepalive=(0, 10, 10, 10, 10, 10, 10, 10, 10, 10, 10, 10, 10, 10, 0, 0),
    wait_out=False,             # skip waiting for the out DMA receipt
)

TRACE = False                   # set by test.py to profile
LAST_RESULTS = None             # BassKernelResults of the last run

_PROG_CACHE = {}


def _build_program_raw(cfg):
    """Raw bacc implementation: manual semaphores, no TileContext."""
    nstrip = cfg["nstrip"]
    w_sched = cfg["w_sched"]
    x_sched = cfg["x_sched"]
    assert sum(c for c, _ in w_sched) == NPAIR
    assert sum(c for c, _ in x_sched) == NPAIR
    n_wc = len(w_sched)
    n_xc = len(x_sched)
    w_start = np.cumsum([0] + [c for c, _ in w_sched])  # pair offsets
    x_start = np.cumsum([0] + [c for c, _ in x_sched])
    # x chunk index needed before starting w chunk c
    x_need = [int(np.searchsorted(x_start, w_start[c + 1], side="left")) - 1
              for c in range(n_wc)]

    first_p = {j: j for j in range(nstrip)}
    last_p = {j: NPAIR - nstrip + j for j in range(nstrip)}

    nc = bacc.Bacc("TRN2", target_bir_lowering=False, debug=False,
                   num_devices=NCORES)
    xt_d = nc.dram_tensor("xt", [KT, NPAIR, 2, MST], mybir.dt.float8e4,
                          kind="ExternalInput")
    wt_d = nc.dram_tensor("wt", [KT, NPAIR, 2, COUT], mybir.dt.float8e4,
                          kind="ExternalInput")
    out_d = nc.dram_tensor("out", [KT, COUT], mybir.dt.float32,
                           kind="ExternalOutput")

    import contextlib
    with contextlib.ExitStack() as stack:
        ec = stack.enter_context
        # one sem per DMA transfer: with several transfers in flight on the
        # 16 SDMA engines, a single cumulative sem is unsound (fast engines
        # can reach 16*(c+1) before a slow engine lands transfer c).
        s_wc = [ec(nc.semaphore(f"s_w{c}")) for c in range(n_wc)]
        s_xc = [ec(nc.semaphore(f"s_x{c}")) for c in range(n_xc)]
        s_mm = ec(nc.semaphore("s_mm"))
        s_cp = ec(nc.semaphore("s_cp"))
        s_out = ec(nc.semaphore("s_out"))
        xs = ec(nc.sbuf_tensor("xs", [KT, NPAIR, 2, MST], mybir.dt.float8e4))
        ws = ec(nc.sbuf_tensor("ws", [KT, NPAIR, 2, COUT], mybir.dt.float8e4))
        osb = ec(nc.sbuf_tensor("osb", [KT, COUT], mybir.dt.float32))
        accs = [ec(nc.psum_tensor(f"acc{s}", [KT, COUT], mybir.dt.float32))
                for s in range(nstrip)]
        if cfg["warmup"] or any(cfg["keepalive"]):
            junk = ec(nc.sbuf_tensor("junk", [KT, COUT], mybir.dt.float8e4))
            scr = ec(nc.psum_tensor("scr", [KT, COUT], mybir.dt.float32))

        # per-ring issue list: (pair_start, kind, chunk_idx); x sorts ahead
        # of w at equal k so the stationary operand is always resident first.
        issues = {0: [], 1: []}
        for c, (chunk, r) in enumerate(x_sched):
            issues[r].append((int(x_start[c]), 0, c))
        for c, (chunk, r) in enumerate(w_sched):
            issues[r].append((int(w_start[c]), 1, c))
        for r in issues:
            issues[r].sort()

        def emit_ring(eng, ring):
            for _p0, kind, c in issues[ring]:
                if kind == 0:
                    a, b = int(x_start[c]), int(x_start[c + 1])
                    eng.dma_start(xs[:, a:b, :, :],
                                  xt_d[:, a:b, :, :]).then_inc(s_xc[c], 16)
                else:
                    a, b = int(w_start[c]), int(w_start[c + 1])
                    eng.dma_start(ws[:, a:b, :, :],
                                  wt_d[:, a:b, :, :]).then_inc(s_wc[c], 16)

        with nc.Block() as block:

            @block.sync
            def _(sync):
                emit_ring(sync, 0)

            @block.scalar
            def _(scalar):
                emit_ring(scalar, 1)
                # tail: evacuate the upper strips in parallel with DVE's,
                # then issue the out DMA right here (scalar is HWDGE too) -
                # saves the cross-engine wake of handing it back to sync.
                scalar.wait_ge(s_mm, 1)
                for s in range((nstrip + 1) // 2, nstrip):
                    scalar.copy(osb[32 * s:32 * s + 32, :],
                                accs[s][32 * s:32 * s + 32, :])
                scalar.wait_ge(s_cp, 1)
                nrow = 32 * nstrip
                scalar.dma_start(out_d[0:nrow, :],
                                 osb[0:nrow, :]).then_inc(s_out, 16)
                if cfg["wait_out"]:
                    scalar.wait_ge(s_out, 16)

            @block.tensor
            def _(tensor):
                def dummy_mms(n):
                    # scratch-bank matmuls: keep the PE busy across DMA waits
                    # so HAM holds the clock up; results are never read
                    for _ in range(n):
                        tensor.matmul(scr[0:B, :], junk[:, 0:B],
                                      junk[:, 0:COUT], start=True, stop=True)

                dummy_mms(cfg["warmup"])
                x_waited = -1
                ka = cfg["keepalive"]
                assert len(ka) == n_wc
                for c, (chunk, _r) in enumerate(w_sched):
                    dummy_mms(ka[c])
                    tensor.wait_ge(s_wc[c], 16)
                    if x_need[c] > x_waited:
                        x_waited = x_need[c]
                        tensor.wait_ge(s_xc[x_waited], 16)
                    for j in range(chunk):
                        p = int(w_start[c]) + j
                        s = p % nstrip
                        mm = tensor.matmul(
                            accs[s][32 * s:32 * s + 32, :],
                            xs[:, p, :, :],
                            ws[:, p, :, :],
                            start=(p == first_p[s]),
                            stop=(p == last_p[s]),
                            perf_mode=mybir.MatmulPerfMode.DoubleRow,
                            tile_position=(0, 32 * s),
                        )
                        if p == NPAIR - 1:
                            mm.then_inc(s_mm, 2)

            @block.vector
            def _(vector):
                vector.wait_ge(s_mm, 1)
                nlow = (nstrip + 1) // 2
                for s in range(nlow):
                    cp = vector.tensor_copy(
                        osb[32 * s:32 * s + 32, :],
                        accs[s][32 * s:32 * s + 32, :],
                    )
                    if s == nlow - 1:
                        cp.then_inc(s_cp, 1)

    nc.compile()
    return nc


def _get_program(cfg):
    key = repr(sorted(cfg.items()))
    if key not in _PROG_CACHE:
        _PROG_CACHE[key] = _build_program_raw(cfg)
    return _PROG_CACHE[key]


def _pack_operand(arr_k_major, ncols):
    """[K_total, ncols] contraction-major -> [KT, NPAIR, 2, ncols] where
    sb[r, p, i, c] = arr[(2p + i)*KT + r, c]."""
    a = arr_k_major.reshape(NKT, KT, ncols).transpose(1, 0, 2)
    return np.ascontiguousarray(a).reshape(KT, NPAIR, 2, ncols)


def kernel(x, weight, bias):
    import os
    if not TRACE:
        # profiling needs an NTFF hook this image lacks; never trace here
        os.environ["BASS_NEVER_TRACE"] = "1"
    else:
        os.environ.pop("BASS_NEVER_TRACE", None)
    x = np.asarray(x, dtype=np.float32)
    weight = np.asarray(weight, dtype=np.float32)
    bias = np.asarray(bias, dtype=np.float32)

    cfg = dict(CFG)
    nc = _get_program(cfg)
    nstrip = cfg["nstrip"]

    # w_rev[o,i,n] = weight[o,i,(L-n) % L]
    idx = (L - np.arange(L)) % L
    wrev = weight[:, :, idx]

    # split x into e4m3 hi + e4m3 residual (DoubleRow needs fp8e4 operands;
    # hi alone would cost 2.6e-2 rel err, hi+residual costs 7.4e-4)
    x_hi8 = x.astype(ml_dtypes.float8_e4m3)
    x_r8 = (x - x_hi8.astype(np.float32)).astype(ml_dtypes.float8_e4m3)

    in_maps = []
    for c in range(NCORES):
        i0 = c * CIN_SH
        wsl = wrev[:, i0:i0 + CIN_SH, :].reshape(COUT, CIN_SH * L)
        wt = _pack_operand(wsl.T.astype(ml_dtypes.float8_e4m3), COUT)
        # [K_total, 32] = hi cols 0-15, residual cols 16-31
        xh = x_hi8[:, i0:i0 + CIN_SH, :].reshape(B, CIN_SH * L).T
        xr = x_r8[:, i0:i0 + CIN_SH, :].reshape(B, CIN_SH * L).T
        xsl = np.concatenate([xh, xr], axis=1)
        xt = _pack_operand(xsl, MST)
        in_maps.append({"xt": xt, "wt": wt})

    global LAST_RESULTS
    res = run_bass_kernel_spmd(nc, in_maps, core_ids=list(range(NCORES)),
                               trace=TRACE)
    LAST_RESULTS = res

    acc = np.zeros((B, COUT), np.float32)
    for c in range(NCORES):
        o = res.results[c]["out"]
        for s in range(nstrip):
            acc += o[32 * s:32 * s + B, :]
            acc += o[32 * s + B:32 * s + 2 * B, :]
    out = acc + bias[None, :]
    return out[:, :, None].astype(np.float32)


# revision 14
# speedup vs baseline: 1.0383x; 1.0057x over previous
"""Trainium2 Bass kernel for nn_Conv1dFFTInt8.

The reference computes, per (b, o):
    out[b,o,0] = ifft(fft(x) . fft(w) summed over cin)[0] + bias[o]
By the circular correlation theorem this collapses to a plain dot product:
    out[b,o] = sum_{i,n} x[b,i,n] * w[o,i,(L-n) % L] + bias[o]

So the whole problem is a GEMM: [B, CIN*L] @ [CIN*L, COUT] with a 524288-deep
contraction. We shard the contraction (CIN) across 8 cores (16 channels each).

v2 (DoubleRow): each core runs 256 fp8e4 DoubleRow matmuls. DoubleRow
processes TWO 128-deep k-tiles per instruction (the PE holds two weight rows
per cell and double-pumps the moving stream), so the PE consumes a pair of
k-tiles in ~53ns instead of ~107ns - the PE drops off the critical path and
the kernel is purely DMA-stream-bound.

DoubleRow requires BOTH operands in fp8e4/e5. x in e4m3 alone costs rel err
2.6e-2 (> the 2e-2 gate), so the stationary operand packs an e4m3 RESIDUAL
correction into the idle M columns: lhsT = [K=128, 2, 32] where cols 0-15
are x_hi = e4m3(x) and cols 16-31 are x_r = e4m3(x - x_hi). One w stream
then computes main + correction simultaneously (PSUM rows 16-31 hold the
correction); host sums the halves. Measured rel err: 7.4e-4.

Streams per core: w 8.39 MB + x 2.10 MB (hi+residual) = 10.5 MB against a
~360-420 GB/s per-core HBM stream, chunked at the 4KB-per-partition
descriptor knee and byte-balanced across the two HWDGE rings (sync/scalar).
Four PE column strips (tile_position cols 0/32/64/96) accumulate in separate
PSUM banks so LDWEIGHTS hides under other strips' matmuls. Warmup dummy MMs
pre-trip the HAM clock up during the (fixed) NEFF preamble; light keepalives
hold it there across chunk waits. Tail: DVE+ACT evacuate strips in parallel,
one 64 KB out DMA whose receipt is not waited (NEFF-end drains cover it).
"""

import numpy as np
import ml_dtypes

import concourse.bass as bass
from concourse import bacc
import concourse.mybir as mybir
from concourse.bass_utils import run_bass_kernel_spmd

B, CIN, COUT, L = 16, 128, 128, 4096
NCORES = 8
CIN_SH = CIN // NCORES          # 16 channels per core
KT = 128                        # contraction depth per k-tile
NKT = CIN_SH * L // KT          # 512 k-tiles per core
NPAIR = NKT // 2                # 256 DoubleRow pairs per core
MST = 2 * B                     # stationary M columns: 16 hi + 16 residual

# --- tunables (A/B config) ---
CFG = dict(
    # DoubleRow ISA restriction (NeuronVerifier check_dual_fp8_restriction):
    # dst.start_partition must be 0, which in bass ties the stationary tile
    # to PE columns 0-31 - so no multi-strip tile_position in DR mode.
    # Multiple PSUM banks at the same partitions do not add LDW overlap, so
    # nstrip stays 1 unless measurement says LDW stalls the pipe.
    nstrip=1,                   # PE column strips (tile_position cols 32*s)
    # w chunk sizes in pairs; 16 pairs = 4KB/partition (the
    # descriptor-efficiency knee). Small tail chunks so the final
    # sem -> last-MM drain is short.
    w_sizes=(16,) * 15 + (8, 8),
    # x chunk sizes in pairs; 64 pairs = 4KB/partition.
    x_sizes=(64,) * 4,
    # Explicit per-ring FIFO issue order (ring 0=sync queue Q1, 1=scalar
    # queue Q10). Measured: Q10's first byte lands ~2.2us after Q1's, and
    # each ring sustains ~half of the ~410 GB/s aggregate, so ring 0 carries
    # more bytes. Orders are arranged so chunk COMPLETION order matches the
    # PE's need order (w0,w1,...), with each x chunk landing well before the
    # w chunks that need it - the previous k-sorted order made ring 1
    # deliver w12 dead last at 40.6us and forced a 3.6us serial PE drain.
    ring0=(("x", 0), ("w", 0), ("w", 2), ("w", 4), ("w", 6), ("x", 2),
           ("w", 8), ("w", 10), ("w", 12), ("w", 14), ("w", 16)),
    ring1=(("x", 1), ("w", 1), ("w", 3), ("w", 5), ("w", 7), ("x", 3),
           ("w", 9), ("w", 11), ("w", 13), ("w", 15)),
    warmup=60,                  # dummy MMs at PE start to pre-trip HAM
    # HAM evaluates PE utilization in ~3.4us windows and halves the clock
    # (k=4) when it sees idle; at k=4 the PE cadence (107ns/pair) falls
    # behind the DMA stream (82ns/pair). Dummies fill the per-chunk waits to
    # hold k=8; none on the tail where the PE may be genuinely behind.
    keepalive=(0, 7, 7, 7, 7, 7, 7, 7, 7, 7, 7, 7, 7, 7, 7, 0, 0),
    out_mode="copy",            # "copy" (bass forbids DMA reads from PSUM)
    wait_out=False,             # skip waiting for the out DMA receipt
)

TRACE = False                   # set by test.py to profile
LAST_RESULTS = None             # BassKernelResults of the last run

_PROG_CACHE = {}


def _build_program_raw(cfg):
    """Raw bacc implementation: manual semaphores, no TileContext."""
    nstrip = cfg["nstrip"]
    w_sizes = cfg["w_sizes"]
    x_sizes = cfg["x_sizes"]
    assert sum(w_sizes) == NPAIR and sum(x_sizes) == NPAIR
    n_wc = len(w_sizes)
    n_xc = len(x_sizes)
    w_start = np.cumsum([0] + list(w_sizes))  # pair offsets
    x_start = np.cumsum([0] + list(x_sizes))
    # x chunk index needed before starting w chunk c
    x_need = [int(np.searchsorted(x_start, w_start[c + 1], side="left")) - 1
              for c in range(n_wc)]
    rings = {0: cfg["ring0"], 1: cfg["ring1"]}
    issued = sorted(t for r in rings.values() for t in r)
    assert issued == sorted([("w", i) for i in range(n_wc)] +
                            [("x", i) for i in range(n_xc)])

    first_p = {j: j for j in range(nstrip)}
    last_p = {j: NPAIR - nstrip + j for j in range(nstrip)}

    nc = bacc.Bacc("TRN2", target_bir_lowering=False, debug=False,
                   num_devices=NCORES)
    xt_d = nc.dram_tensor("xt", [KT, NPAIR, 2, MST], mybir.dt.float8e4,
                          kind="ExternalInput")
    wt_d = nc.dram_tensor("wt", [KT, NPAIR, 2, COUT], mybir.dt.float8e4,
                          kind="ExternalInput")
    out_d = nc.dram_tensor("out", [KT, COUT], mybir.dt.float32,
                           kind="ExternalOutput")

    import contextlib
    with contextlib.ExitStack() as stack:
        ec = stack.enter_context
        # one sem per DMA transfer: with several transfers in flight on the
        # 16 SDMA engines, a single cumulative sem is unsound (fast engines
        # can reach 16*(c+1) before a slow engine lands transfer c).
        s_wc = [ec(nc.semaphore(f"s_w{c}")) for c in range(n_wc)]
        s_xc = [ec(nc.semaphore(f"s_x{c}")) for c in range(n_xc)]
        s_mm = ec(nc.semaphore("s_mm"))
        s_out = ec(nc.semaphore("s_out"))
        xs = ec(nc.sbuf_tensor("xs", [KT, NPAIR, 2, MST], mybir.dt.float8e4))
        ws = ec(nc.sbuf_tensor("ws", [KT, NPAIR, 2, COUT], mybir.dt.float8e4))
        osb = ec(nc.sbuf_tensor("osb", [KT, COUT], mybir.dt.float32))
        accs = [ec(nc.psum_tensor(f"acc{s}", [KT, COUT], mybir.dt.float32))
                for s in range(nstrip)]
        if cfg["warmup"] or any(cfg["keepalive"]):
            junk = ec(nc.sbuf_tensor("junk", [KT, COUT], mybir.dt.float8e4))
            scr = ec(nc.psum_tensor("scr", [KT, COUT], mybir.dt.float32))

        def emit_ring(eng, ring):
            for kind, c in rings[ring]:
                if kind == "x":
                    a, b = int(x_start[c]), int(x_start[c + 1])
                    eng.dma_start(xs[:, a:b, :, :],
                                  xt_d[:, a:b, :, :]).then_inc(s_xc[c], 16)
                else:
                    a, b = int(w_start[c]), int(w_start[c + 1])
                    eng.dma_start(ws[:, a:b, :, :],
                                  wt_d[:, a:b, :, :]).then_inc(s_wc[c], 16)

        with nc.Block() as block:

            @block.sync
            def _(sync):
                emit_ring(sync, 0)

            @block.scalar
            def _(scalar):
                emit_ring(scalar, 1)
                # tail: single-engine out path (scalar is HWDGE) - no
                # cross-engine handoffs after the last matmul.
                scalar.wait_ge(s_mm, 1)
                nrow = 32 * nstrip
                if cfg["out_mode"] == "psum_direct":
                    scalar.dma_start(out_d[0:nrow, :],
                                     accs[0][0:nrow, :]).then_inc(s_out, 16)
                else:
                    scalar.copy(osb[0:nrow, :], accs[0][0:nrow, :])
                    scalar.dma_start(out_d[0:nrow, :],
                                     osb[0:nrow, :]).then_inc(s_out, 16)
                if cfg["wait_out"]:
                    scalar.wait_ge(s_out, 16)

            @block.tensor
            def _(tensor):
                def dummy_mms(n):
                    # scratch-bank matmuls: keep the PE busy across DMA waits
                    # so HAM holds the clock up; results are never read
                    for _ in range(n):
                        tensor.matmul(scr[0:B, :], junk[:, 0:B],
                                      junk[:, 0:COUT], start=True, stop=True)

                dummy_mms(cfg["warmup"])
                x_waited = -1
                ka = cfg["keepalive"]
                assert len(ka) == n_wc
                for c, chunk in enumerate(w_sizes):
                    dummy_mms(ka[c])
                    tensor.wait_ge(s_wc[c], 16)
                    if x_need[c] > x_waited:
                        x_waited = x_need[c]
                        tensor.wait_ge(s_xc[x_waited], 16)
                    for j in range(chunk):
                        p = int(w_start[c]) + j
                        s = p % nstrip
                        mm = tensor.matmul(
                            accs[s][32 * s:32 * s + 32, :],
                            xs[:, p, :, :],
                            ws[:, p, :, :],
                            start=(p == first_p[s]),
                            stop=(p == last_p[s]),
                            perf_mode=mybir.MatmulPerfMode.DoubleRow,
                            tile_position=(0, 32 * s),
                        )
                        if p == NPAIR - 1:
                            mm.then_inc(s_mm, 2)



    nc.compile()
    return nc


def _get_program(cfg):
    key = repr(sorted(cfg.items()))
    if key not in _PROG_CACHE:
        _PROG_CACHE[key] = _build_program_raw(cfg)
    return _PROG_CACHE[key]


def _pack_operand(arr_k_major, ncols):
    """[K_total, ncols] contraction-major -> [KT, NPAIR, 2, ncols] where
    sb[r, p, i, c] = arr[(2p + i)*KT + r, c]."""
    a = arr_k_major.reshape(NKT, KT, ncols).transpose(1, 0, 2)
    return np.ascontiguousarray(a).reshape(KT, NPAIR, 2, ncols)


def kernel(x, weight, bias):
    import os
    if not TRACE:
        # profiling needs an NTFF hook this image lacks; never trace here
        os.environ["BASS_NEVER_TRACE"] = "1"
    else:
        os.environ.pop("BASS_NEVER_TRACE", None)
    x = np.asarray(x, dtype=np.float32)
    weight = np.asarray(weight, dtype=np.float32)
    bias = np.asarray(bias, dtype=np.float32)

    cfg = dict(CFG)
    nc = _get_program(cfg)
    nstrip = cfg["nstrip"]

    # w_rev[o,i,n] = weight[o,i,(L-n) % L]
    idx = (L - np.arange(L)) % L
    wrev = weight[:, :, idx]

    # split x into e4m3 hi + e4m3 residual (DoubleRow needs fp8e4 operands;
    # hi alone would cost 2.6e-2 rel err, hi+residual costs 7.4e-4)
    x_hi8 = x.astype(ml_dtypes.float8_e4m3)
    x_r8 = (x - x_hi8.astype(np.float32)).astype(ml_dtypes.float8_e4m3)

    in_maps = []
    for c in range(NCORES):
        i0 = c * CIN_SH
        wsl = wrev[:, i0:i0 + CIN_SH, :].reshape(COUT, CIN_SH * L)
        wt = _pack_operand(wsl.T.astype(ml_dtypes.float8_e4m3), COUT)
        # [K_total, 32] = hi cols 0-15, residual cols 16-31
        xh = x_hi8[:, i0:i0 + CIN_SH, :].reshape(B, CIN_SH * L).T
        xr = x_r8[:, i0:i0 + CIN_SH, :].reshape(B, CIN_SH * L).T
        xsl = np.concatenate([xh, xr], axis=1)
        xt = _pack_operand(xsl, MST)
        in_maps.append({"xt": xt, "wt": wt})

    global LAST_RESULTS
    res = run_bass_kernel_spmd(nc, in_maps, core_ids=list(range(NCORES)),
                               trace=TRACE)
    LAST_RESULTS = res

    acc = np.zeros((B, COUT), np.float32)
    for c in range(NCORES):
        o = res.results[c]["out"]
        for s in range(nstrip):
            acc += o[32 * s:32 * s + B, :]
            acc += o[32 * s + B:32 * s + 2 * B, :]
    out = acc + bias[None, :]
    return out[:, :, None].astype(np.float32)


# revision 15
# speedup vs baseline: 1.1132x; 1.0722x over previous
"""Trainium2 Bass kernel for nn_Conv1dFFTInt8.

The reference computes, per (b, o):
    out[b,o,0] = ifft(fft(x) . fft(w) summed over cin)[0] + bias[o]
By the circular correlation theorem this collapses to a plain dot product:
    out[b,o] = sum_{i,n} x[b,i,n] * w[o,i,(L-n) % L] + bias[o]

So the whole problem is a GEMM: [B, CIN*L] @ [CIN*L, COUT] with a 524288-deep
contraction. We shard the contraction (CIN) across 8 cores (16 channels each).

v2 (DoubleRow): each core runs 256 fp8e4 DoubleRow matmuls. DoubleRow
processes TWO 128-deep k-tiles per instruction (the PE holds two weight rows
per cell and double-pumps the moving stream), so the PE consumes a pair of
k-tiles in ~53ns instead of ~107ns - the PE drops off the critical path and
the kernel is purely DMA-stream-bound.

DoubleRow requires BOTH operands in fp8e4/e5. x in e4m3 alone costs rel err
2.6e-2 (> the 2e-2 gate), so the stationary operand packs an e4m3 RESIDUAL
correction into the idle M columns: lhsT = [K=128, 2, 32] where cols 0-15
are x_hi = e4m3(x) and cols 16-31 are x_r = e4m3(x - x_hi). One w stream
then computes main + correction simultaneously (PSUM rows 16-31 hold the
correction); host sums the halves. Measured rel err: 7.4e-4.

Streams per core: w 8.39 MB + x 2.10 MB (hi+residual) = 10.5 MB against a
~360-420 GB/s per-core HBM stream, chunked at the 4KB-per-partition
descriptor knee and byte-balanced across the two HWDGE rings (sync/scalar).
Four PE column strips (tile_position cols 0/32/64/96) accumulate in separate
PSUM banks so LDWEIGHTS hides under other strips' matmuls. Warmup dummy MMs
pre-trip the HAM clock up during the (fixed) NEFF preamble; light keepalives
hold it there across chunk waits. Tail: DVE+ACT evacuate strips in parallel,
one 64 KB out DMA whose receipt is not waited (NEFF-end drains cover it).
"""

import numpy as np
import ml_dtypes

import concourse.bass as bass
from concourse import bacc
import concourse.mybir as mybir
from concourse.bass_utils import run_bass_kernel_spmd

B, CIN, COUT, L = 16, 128, 128, 4096
NCORES = 8
CIN_SH = CIN // NCORES          # 16 channels per core
KT = 128                        # contraction depth per k-tile
NKT = CIN_SH * L // KT          # 512 k-tiles per core
NPAIR = NKT // 2                # 256 DoubleRow pairs per core
MST = 2 * B                     # stationary M columns: 16 hi + 16 residual

# --- tunables (A/B config) ---
CFG = dict(
    # DoubleRow ISA restriction (NeuronVerifier check_dual_fp8_restriction):
    # dst.start_partition must be 0, which in bass ties the stationary tile
    # to PE columns 0-31 - so no multi-strip tile_position in DR mode.
    # Multiple PSUM banks at the same partitions do not add LDW overlap, so
    # nstrip stays 1 unless measurement says LDW stalls the pipe.
    nstrip=1,                   # PE column strips (tile_position cols 32*s)
    # w chunk sizes in pairs; 16 pairs = 4KB/partition (the
    # descriptor-efficiency knee). Small tail chunks so the final
    # sem -> last-MM drain is short.
    w_sizes=(16,) * 15 + (8, 8),
    # x chunk sizes in pairs; 32 pairs = 2KB/partition, interleaved
    # just-in-time so no big x transfer front-loads a ring.
    x_sizes=(32,) * 8,
    # Explicit per-ring FIFO issue order (ring 0=sync queue Q1, 1=scalar
    # queue Q10). Measured: Q10's first byte lands ~2.2us after Q1's, and
    # each ring sustains ~half of the ~410 GB/s aggregate, so ring 0 carries
    # more bytes. Orders are arranged so chunk COMPLETION order matches the
    # PE's need order (w0,w1,...), with each x chunk landing 2-3 chunks
    # before the w chunks that need it - a k-sorted order once made ring 1
    # deliver w12 dead last at 40.6us and forced a 3.6us serial PE drain.
    ring0=(("x", 0), ("w", 0), ("x", 2), ("w", 2), ("x", 4), ("w", 4),
           ("x", 6), ("w", 6), ("w", 8), ("w", 10), ("w", 12), ("w", 14),
           ("w", 16)),
    ring1=(("x", 1), ("w", 1), ("x", 3), ("w", 3), ("x", 5), ("w", 5),
           ("x", 7), ("w", 7), ("w", 9), ("w", 11), ("w", 13), ("w", 15)),
    warmup=45,                  # dummy MMs at PE start to pre-trip HAM and
                                # ride out the p-state ramp; ends right as
                                # w0's data lands
    # The PE runs at ~86% natural utilization against the stream (16-pair
    # chunk = 1.07us of DR matmuls per ~1.25us sem cadence), enough to hold
    # HAM at k=8 without dummy fill - and every dummy issued while the PE is
    # behind adds straight to the critical path (measured: the PE ran
    # gapless 36->44us draining dummies+backlog). So no mid-stream dummies.
    keepalive=(0,) * 17,
    out_mode="copy",            # "copy" (bass forbids DMA reads from PSUM)
    wait_out=False,             # skip waiting for the out DMA receipt
)

TRACE = False                   # set by test.py to profile
LAST_RESULTS = None             # BassKernelResults of the last run

_PROG_CACHE = {}


def _build_program_raw(cfg):
    """Raw bacc implementation: manual semaphores, no TileContext."""
    nstrip = cfg["nstrip"]
    w_sizes = cfg["w_sizes"]
    x_sizes = cfg["x_sizes"]
    assert sum(w_sizes) == NPAIR and sum(x_sizes) == NPAIR
    n_wc = len(w_sizes)
    n_xc = len(x_sizes)
    w_start = np.cumsum([0] + list(w_sizes))  # pair offsets
    x_start = np.cumsum([0] + list(x_sizes))
    # x chunk index needed before starting w chunk c
    x_need = [int(np.searchsorted(x_start, w_start[c + 1], side="left")) - 1
              for c in range(n_wc)]
    rings = {0: cfg["ring0"], 1: cfg["ring1"]}
    issued = sorted(t for r in rings.values() for t in r)
    assert issued == sorted([("w", i) for i in range(n_wc)] +
                            [("x", i) for i in range(n_xc)])

    first_p = {j: j for j in range(nstrip)}
    last_p = {j: NPAIR - nstrip + j for j in range(nstrip)}

    nc = bacc.Bacc("TRN2", target_bir_lowering=False, debug=False,
                   num_devices=NCORES)
    xt_d = nc.dram_tensor("xt", [KT, NPAIR, 2, MST], mybir.dt.float8e4,
                          kind="ExternalInput")
    wt_d = nc.dram_tensor("wt", [KT, NPAIR, 2, COUT], mybir.dt.float8e4,
                          kind="ExternalInput")
    out_d = nc.dram_tensor("out", [KT, COUT], mybir.dt.float32,
                           kind="ExternalOutput")

    import contextlib
    with contextlib.ExitStack() as stack:
        ec = stack.enter_context
        # one sem per DMA transfer: with several transfers in flight on the
        # 16 SDMA engines, a single cumulative sem is unsound (fast engines
        # can reach 16*(c+1) before a slow engine lands transfer c).
        s_wc = [ec(nc.semaphore(f"s_w{c}")) for c in range(n_wc)]
        s_xc = [ec(nc.semaphore(f"s_x{c}")) for c in range(n_xc)]
        s_mm = ec(nc.semaphore("s_mm"))
        s_out = ec(nc.semaphore("s_out"))
        xs = ec(nc.sbuf_tensor("xs", [KT, NPAIR, 2, MST], mybir.dt.float8e4))
        ws = ec(nc.sbuf_tensor("ws", [KT, NPAIR, 2, COUT], mybir.dt.float8e4))
        osb = ec(nc.sbuf_tensor("osb", [KT, COUT], mybir.dt.float32))
        accs = [ec(nc.psum_tensor(f"acc{s}", [KT, COUT], mybir.dt.float32))
                for s in range(nstrip)]
        if cfg["warmup"] or any(cfg["keepalive"]):
            junk = ec(nc.sbuf_tensor("junk", [KT, COUT], mybir.dt.float8e4))
            scr = ec(nc.psum_tensor("scr", [KT, COUT], mybir.dt.float32))

        def emit_ring(eng, ring):
            for kind, c in rings[ring]:
                if kind == "x":
                    a, b = int(x_start[c]), int(x_start[c + 1])
                    eng.dma_start(xs[:, a:b, :, :],
                                  xt_d[:, a:b, :, :]).then_inc(s_xc[c], 16)
                else:
                    a, b = int(w_start[c]), int(w_start[c + 1])
                    eng.dma_start(ws[:, a:b, :, :],
                                  wt_d[:, a:b, :, :]).then_inc(s_wc[c], 16)

        with nc.Block() as block:

            @block.sync
            def _(sync):
                emit_ring(sync, 0)

            @block.scalar
            def _(scalar):
                emit_ring(scalar, 1)
                # tail: single-engine out path (scalar is HWDGE) - no
                # cross-engine handoffs after the last matmul.
                scalar.wait_ge(s_mm, 1)
                nrow = 32 * nstrip
                if cfg["out_mode"] == "psum_direct":
                    scalar.dma_start(out_d[0:nrow, :],
                                     accs[0][0:nrow, :]).then_inc(s_out, 16)
                else:
                    scalar.copy(osb[0:nrow, :], accs[0][0:nrow, :])
                    scalar.dma_start(out_d[0:nrow, :],
                                     osb[0:nrow, :]).then_inc(s_out, 16)
                if cfg["wait_out"]:
                    scalar.wait_ge(s_out, 16)

            @block.tensor
            def _(tensor):
                def dummy_mms(n):
                    # scratch-bank matmuls: keep the PE busy across DMA waits
                    # so HAM holds the clock up; results are never read
                    for _ in range(n):
                        tensor.matmul(scr[0:B, :], junk[:, 0:B],
                                      junk[:, 0:COUT], start=True, stop=True)

                dummy_mms(cfg["warmup"])
                x_waited = -1
                ka = cfg["keepalive"]
                assert len(ka) == n_wc
                for c, chunk in enumerate(w_sizes):
                    dummy_mms(ka[c])
                    tensor.wait_ge(s_wc[c], 16)
                    if x_need[c] > x_waited:
                        x_waited = x_need[c]
                        tensor.wait_ge(s_xc[x_waited], 16)
                    for j in range(chunk):
                        p = int(w_start[c]) + j
                        s = p % nstrip
                        mm = tensor.matmul(
                            accs[s][32 * s:32 * s + 32, :],
                            xs[:, p, :, :],
                            ws[:, p, :, :],
                            start=(p == first_p[s]),
                            stop=(p == last_p[s]),
                            perf_mode=mybir.MatmulPerfMode.DoubleRow,
                            tile_position=(0, 32 * s),
                        )
                        if p == NPAIR - 1:
                            mm.then_inc(s_mm, 2)



    nc.compile()
    return nc


def _get_program(cfg):
    key = repr(sorted(cfg.items()))
    if key not in _PROG_CACHE:
        _PROG_CACHE[key] = _build_program_raw(cfg)
    return _PROG_CACHE[key]


def _pack_operand(arr_k_major, ncols):
    """[K_total, ncols] contraction-major -> [KT, NPAIR, 2, ncols] where
    sb[r, p, i, c] = arr[(2p + i)*KT + r, c]."""
    a = arr_k_major.reshape(NKT, KT, ncols).transpose(1, 0, 2)
    return np.ascontiguousarray(a).reshape(KT, NPAIR, 2, ncols)


def kernel(x, weight, bias):
    import os
    if not TRACE:
        # profiling needs an NTFF hook this image lacks; never trace here
        os.environ["BASS_NEVER_TRACE"] = "1"
    else:
        os.environ.pop("BASS_NEVER_TRACE", None)
    x = np.asarray(x, dtype=np.float32)
    weight = np.asarray(weight, dtype=np.float32)
    bias = np.asarray(bias, dtype=np.float32)

    cfg = dict(CFG)
    nc = _get_program(cfg)
    nstrip = cfg["nstrip"]

    # w_rev[o,i,n] = weight[o,i,(L-n) % L]
    idx = (L - np.arange(L)) % L
    wrev = weight[:, :, idx]

    # split x into e4m3 hi + e4m3 residual (DoubleRow needs fp8e4 operands;
    # hi alone would cost 2.6e-2 rel err, hi+residual costs 7.4e-4)
    x_hi8 = x.astype(ml_dtypes.float8_e4m3)
    x_r8 = (x - x_hi8.astype(np.float32)).astype(ml_dtypes.float8_e4m3)

    in_maps = []
    for c in range(NCORES):
        i0 = c * CIN_SH
        wsl = wrev[:, i0:i0 + CIN_SH, :].reshape(COUT, CIN_SH * L)
        wt = _pack_operand(wsl.T.astype(ml_dtypes.float8_e4m3), COUT)
        # [K_total, 32] = hi cols 0-15, residual cols 16-31
        xh = x_hi8[:, i0:i0 + CIN_SH, :].reshape(B, CIN_SH * L).T
        xr = x_r8[:, i0:i0 + CIN_SH, :].reshape(B, CIN_SH * L).T
        xsl = np.concatenate([xh, xr], axis=1)
        xt = _pack_operand(xsl, MST)
        in_maps.append({"xt": xt, "wt": wt})

    global LAST_RESULTS
    res = run_bass_kernel_spmd(nc, in_maps, core_ids=list(range(NCORES)),
                               trace=TRACE)
    LAST_RESULTS = res

    acc = np.zeros((B, COUT), np.float32)
    for c in range(NCORES):
        o = res.results[c]["out"]
        for s in range(nstrip):
            acc += o[32 * s:32 * s + B, :]
            acc += o[32 * s + B:32 * s + 2 * B, :]
    out = acc + bias[None, :]
    return out[:, :, None].astype(np.float32)
